# revision 1
# baseline (speedup 1.0000x reference)
"""Causal multi-head attention (B=2, T=2048, C=1024, H=16) on 8 Trainium2 cores.

Sharding: batch x head-group. Core c handles batch b = c//4 and heads
[4*(c%4), 4*(c%4)+4).  Each core computes its 4 heads' QKV projections
(tensor-parallel column split), flash-style causal attention in transposed
layout (scores kept as S^T[k, q] so both the QK^T and the PV matmuls run
without any transposes), and a partial output projection over its 256
attention channels.  The 4 partial [T, C] projections per batch are summed on
the host (the contraction over heads), and the bias is added there too.

Matmuls run in float32r (single-pass fp32, ~tf32 accuracy, 4x faster than
exact fp32 on the PE); softmax statistics (l = row sums) ride along as a
65th "ones" column of V, so no separate reduction pass is needed.
"""

import sys

sys.path.insert(0, "/opt/trn_rl_repo")

import numpy as np

import concourse.bass as bass  # noqa: F401  (import registers AP machinery)
import concourse.mybir as mybir
import concourse.tile as tile
from concourse import bacc

F32 = mybir.dt.float32
F32R = mybir.dt.float32r
EXP = mybir.ActivationFunctionType.Exp
IS_GE = mybir.AluOpType.is_ge

B = 2
C = 1024
NH = 16
D = 64
HS = 256          # head-slice channels per core (4 heads x 64)
NCORES = 8
NKC = C // 128    # contraction chunks for the projections


def build_nc(T=2048, debug_dump=False):
    """Build the per-core Bass program (same program on all 8 cores)."""
    NQW = T // 512    # 512-wide q windows
    NTB = T // 128    # 128-row t blocks
    SCALE = 1.0 / np.sqrt(D)

    nc = bacc.Bacc("TRN2", target_bir_lowering=False, debug=False,
                   num_devices=NCORES)

    xT = nc.dram_tensor("xT", [C, T], F32R, kind="ExternalInput").ap()
    wqT = nc.dram_tensor("wqT", [C, HS], F32R, kind="ExternalInput").ap()
    wkT = nc.dram_tensor("wkT", [C, HS], F32R, kind="ExternalInput").ap()
    wvT = nc.dram_tensor("wvT", [C, HS], F32R, kind="ExternalInput").ap()
    wpT = nc.dram_tensor("wpT", [HS, C], F32R, kind="ExternalInput").ap()
    out = nc.dram_tensor("out", [T, C], F32, kind="ExternalOutput").ap()
    dbg = {}
    if debug_dump:
        for nm, shp in (("d_qt0", [128, T]), ("d_kt0", [128, T]),
                        ("d_v0", [128, 260]), ("d_ot0", [128, T]),
                        ("d_ot1", [128, T]), ("d_se", [128, 1024]),
                        ("d_lbs", [64, 1024]), ("d_av", [65, 512])):
            dbg[nm] = nc.dram_tensor(nm, shp, F32, kind="ExternalOutput").ap()

    with tile.TileContext(nc) as tc:
        with (
            tc.tile_pool(name="pers", bufs=1) as pers,
            tc.tile_pool(name="psst", bufs=2, space="PSUM") as psst,
            tc.tile_pool(name="psav", bufs=3, space="PSUM") as psav,
            tc.tile_pool(name="pspj", bufs=1, space="PSUM") as pspj,
            tc.tile_pool(name="sework", bufs=3) as sework,
            tc.tile_pool(name="outw", bufs=3) as outw,
            tc.tile_pool(name="tmpw", bufs=2) as tmpw,
        ):
            xt_sb = [pers.tile([128, T], F32R, tag=f"xt{c}", name=f"xt{c}") for c in range(NKC)]
            wq_sb = [pers.tile([128, HS], F32R, tag=f"wq{c}", name=f"wq{c}") for c in range(NKC)]
            wk_sb = [pers.tile([128, HS], F32R, tag=f"wk{c}", name=f"wk{c}") for c in range(NKC)]
            wv_sb = [pers.tile([128, HS], F32R, tag=f"wv{c}", name=f"wv{c}") for c in range(NKC)]
            wp_sb = [pers.tile([128, C], F32R, tag=f"wp{cc}", name=f"wp{cc}") for cc in range(2)]
            # per-window Q^T/K^T/OT tiles so attention/proj can start before the
            # whole projection phase finishes (fine-grained dependencies)
            qt_sb = [[pers.tile([128, 512], F32R, tag=f"qt{m}_{w}", name=f"qt{m}_{w}")
                      for w in range(NQW)] for m in range(2)]
            kt_sb = [[pers.tile([128, 512], F32R, tag=f"kt{m}_{w}", name=f"kt{m}_{w}")
                      for w in range(NQW)] for m in range(2)]
            v_sb = [pers.tile([128, 4 * 65], F32R, tag=f"v{tb}", name=f"v{tb}") for tb in range(NTB)]
            ot_sb = [[pers.tile([128, 512], F32R, tag=f"ot{cc}_{w}", name=f"ot{cc}_{w}")
                      for w in range(NQW)] for cc in range(2)]
            ones_f = pers.tile([65, 64], F32, tag="ones_f", name="ones_f")
            ones_r = pers.tile([65, 64], F32R, tag="ones_r", name="ones_r")
            onesc_f = pers.tile([128, 4], F32, tag="onesc_f", name="onesc_f")

            # ---- input loads ----
            for c in range(NKC):
                nc.sync.dma_start(out=xt_sb[c][:, :], in_=xT[c * 128:(c + 1) * 128, :])
                nc.sync.dma_start(out=wq_sb[c][:, :], in_=wqT[c * 128:(c + 1) * 128, :])
                nc.sync.dma_start(out=wk_sb[c][:, :], in_=wkT[c * 128:(c + 1) * 128, :])
                nc.sync.dma_start(out=wv_sb[c][:, :], in_=wvT[c * 128:(c + 1) * 128, :])
            for cc in range(2):
                nc.sync.dma_start(out=wp_sb[cc][:, :], in_=wpT[cc * 128:(cc + 1) * 128, :])
            nc.gpsimd.memset(ones_f[:, :], 1.0)
            nc.vector.tensor_copy(ones_r[64:65, :], ones_f[64:65, :])
            nc.gpsimd.memset(onesc_f[:, :], 1.0)

            # ---- QKV projections ----
            # V first (AV matmuls need it earliest), natural [t, cout] layout.
            for tb in range(NTB):
                p = psst.tile([128, 1024], F32, tag="st", name="st")
                for c in range(NKC):
                    nc.tensor.matmul(
                        p[:, 0:HS],
                        xt_sb[c][:, tb * 128:(tb + 1) * 128],
                        wv_sb[c][:, :],
                        start=(c == 0), stop=(c == NKC - 1),
                    )
                vdst = v_sb[tb][:, :].rearrange("p (h c) -> p h c", h=4)
                vsrc = p[:, 0:HS].rearrange("p (h c) -> p h c", h=4)
                nc.any.tensor_copy(vdst[:, :, 0:64], vsrc)
                nc.any.tensor_copy(
                    vdst[:, :, 64:65],
                    onesc_f[:, :].rearrange("p (h o) -> p h o", o=1),
                )
            # Q^T / K^T in [cout, t] layout: lhsT = W^T chunk, rhs = x^T chunk.
            for w in range(NQW):
                for w_sb, dst in ((wk_sb, kt_sb), (wq_sb, qt_sb)):
                    for m in range(2):
                        p = psst.tile([128, 1024], F32, tag="st", name="st")
                        for c in range(NKC):
                            nc.tensor.matmul(
                                p[:, 0:512],
                                w_sb[c][:, m * 128:(m + 1) * 128],
                                xt_sb[c][:, w * 512:(w + 1) * 512],
                                start=(c == 0), stop=(c == NKC - 1),
                            )
                        nc.any.tensor_copy(dst[m][w][:, :], p[:, 0:512])
            if debug_dump:
                for w in range(NQW):
                    nc.sync.dma_start(out=dbg["d_qt0"][:, w * 512:(w + 1) * 512],
                                      in_=qt_sb[0][w][:, :].bitcast(F32))
                    nc.sync.dma_start(out=dbg["d_kt0"][:, w * 512:(w + 1) * 512],
                                      in_=kt_sb[0][w][:, :].bitcast(F32))
                nc.sync.dma_start(out=dbg["d_v0"][:, :], in_=v_sb[0][:, :].bitcast(F32))

            # ---- attention (flash-style, transposed scores S^T[k, q]) ----
            for qw in range(NQW):
                for cc in range(2):       # head pair (2*cc, 2*cc+1)
                    q0 = qw * 512
                    avA = psav.tile([65, 512], F32, tag="av", name="av")
                    avB = psav.tile([65, 512], F32, tag="av", name="av")
                    nkb = 4 * qw + 4      # causal: k blocks up to the diagonal
                    for kb in range(nkb):
                        k0 = kb * 128
                        stp = psst.tile([128, 1024], F32, tag="st", name="st")
                        kw, kcol = kb // 4, (k0 % 512)
                        nc.tensor.matmul(
                            stp[:, 0:512],
                            kt_sb[cc][kw][0:64, kcol:kcol + 128],
                            qt_sb[cc][qw][0:64, :],
                            start=True, stop=True,
                        )
                        nc.tensor.matmul(
                            stp[:, 512:1024],
                            kt_sb[cc][kw][64:128, kcol:kcol + 128],
                            qt_sb[cc][qw][64:128, :],
                            start=True, stop=True,
                        )
                        sep = sework.tile([128, 1024], F32R, tag="se", name="se")
                        j = kb - 4 * qw
                        if j < 0:
                            nc.scalar.activation(sep[:, :], stp[:, :], EXP, scale=SCALE)
                        else:
                            # diagonal stripe: cols < 128j fully masked,
                            # cols [128j, 128j+128) triangular, rest kept.
                            if j > 0:
                                z = sep[:, :].rearrange(
                                    "p (h q) -> p h q", h=2)[:, :, 0:128 * j]
                                nc.gpsimd.memset(z.bitcast(F32), 0.0)
                            src3 = stp[:, :].rearrange(
                                "p (h q) -> p h q", h=2)[:, :, 128 * j:512]
                            dst3 = sep[:, :].rearrange(
                                "p (h q) -> p h q", h=2)[:, :, 128 * j:512]
                            nc.scalar.activation(dst3, src3, EXP, scale=SCALE)
                            for hh in range(2):
                                sl = sep[:, hh * 512 + 128 * j:
                                         hh * 512 + 128 * j + 128]
                                nc.gpsimd.affine_select(
                                    out=sl, in_=sl, compare_op=IS_GE, fill=0.0,
                                    base=0, pattern=[[1, 128]],
                                    channel_multiplier=-1,
                                )
                        hA, hB = 2 * cc, 2 * cc + 1
                        lA = hA - 2 * cc      # 0
                        lB = hB - 2 * cc      # 1
                        nc.tensor.matmul(
                            avA[:, :], v_sb[kb][:, (2 * cc) * 65:(2 * cc) * 65 + 65],
                            sep[:, 0:512],
                            start=(kb == 0), stop=(kb == nkb - 1),
                        )
                        nc.tensor.matmul(
                            avB[:, :], v_sb[kb][:, (2 * cc + 1) * 65:(2 * cc + 1) * 65 + 65],
                            sep[:, 512:1024],
                            start=(kb == 0), stop=(kb == nkb - 1),
                        )
                        if debug_dump and cc == 0 and qw == NQW - 1 and kb == nkb - 1:
                            nc.sync.dma_start(out=dbg["d_se"][:, :],
                                              in_=sep[:, :].bitcast(F32))
                    # normalization: rows 0..63 = V^T @ S^T, row 64 = l (sums)
                    linvf = tmpw.tile([65, 1024], F32, tag="linvf", name="linvf")
                    linvr = tmpw.tile([65, 1024], F32R, tag="linvr", name="linvr")
                    # NOTE: reciprocal_approx_fast misbehaves on HW for APs
                    # whose base partition is 64; run it over rows 0..64
                    # (base 0) and use only row 64. Rows 0..63 are garbage
                    # reciprocals of attention numerators and never read.
                    nc.vector.reciprocal_approx_fast(
                        out=linvf[0:65, 0:512], in_=avA[0:65, :])
                    nc.vector.reciprocal_approx_fast(
                        out=linvf[0:65, 512:1024], in_=avB[0:65, :])
                    nc.vector.tensor_copy(linvr[64:65, :], linvf[64:65, :])
                    lbpA = psav.tile([65, 512], F32, tag="av", name="av")
                    lbpB = psav.tile([65, 512], F32, tag="av", name="av")
                    nc.tensor.matmul(lbpA[0:64, :], ones_r[64:65, :],
                                     linvr[64:65, 0:512], start=True, stop=True)
                    nc.tensor.matmul(lbpB[0:64, :], ones_r[64:65, :],
                                     linvr[64:65, 512:1024], start=True, stop=True)
                    lbs = tmpw.tile([64, 1024], F32, tag="lbs", name="lbs")
                    nc.vector.tensor_copy(lbs[:, 0:512], lbpA[0:64, :])
                    nc.vector.tensor_copy(lbs[:, 512:1024], lbpB[0:64, :])
                    # head A (even) lands on OT rows 0..63 directly
                    nc.vector.tensor_mul(ot_sb[cc][qw][0:64, :],
                                         avA[0:64, :], lbs[:, 0:512])
                    # head B (odd) needs a partition shift to OT rows 64..127
                    tmp = tmpw.tile([64, 512], F32R, tag="tmp", name="tmp")
                    nc.vector.tensor_mul(tmp[:, :], avB[0:64, :], lbs[:, 512:1024])
                    nc.sync.dma_start(out=ot_sb[cc][qw][64:128, :],
                                      in_=tmp[:, :])
                    if debug_dump and cc == 0 and qw == NQW - 1:
                        nc.sync.dma_start(out=dbg["d_lbs"][:, :], in_=lbs[:, :])
                        avd = outw.tile([65, 512], F32, tag="avd", name="avd")
                        nc.vector.tensor_copy(avd[:, :], avA[:, :])
                        nc.sync.dma_start(out=dbg["d_av"][:, :], in_=avd[:, :])

                # ---- output projection for this q-window's four t-blocks ----
                for tb in range(4 * qw, 4 * qw + 4):
                    for nw in range(C // 512):
                        p = pspj.tile([128, 512], F32, tag="pj", name="pj")
                        for cc2 in range(2):
                            nc.tensor.matmul(
                                p[:, 0:512],
                                ot_sb[cc2][qw][:, (tb % 4) * 128:(tb % 4) * 128 + 128],
                                wp_sb[cc2][:, nw * 512:(nw + 1) * 512],
                                start=(cc2 == 0), stop=(cc2 == 1),
                            )
                        so = outw.tile([128, 512], F32, tag="so", name="so")
                        nc.any.tensor_copy(so[:, :], p[:, 0:512])
                        nc.sync.dma_start(
                            out=out[tb * 128:(tb + 1) * 128, nw * 512:(nw + 1) * 512],
                            in_=so[:, :])
            if debug_dump:
                for w in range(NQW):
                    nc.sync.dma_start(out=dbg["d_ot0"][:, w * 512:(w + 1) * 512],
                                      in_=ot_sb[0][w][:, :].bitcast(F32))
                    nc.sync.dma_start(out=dbg["d_ot1"][:, w * 512:(w + 1) * 512],
                                      in_=ot_sb[1][w][:, :].bitcast(F32))

    nc.finalize()
    return nc


# ---------------------------------------------------------------------------
# host-side runner with a cached jitted executable (compile once per process)
# ---------------------------------------------------------------------------

_RUNNERS = {}


class _Runner:
    def __init__(self, T=2048, debug_dump=False):
        import os
        import jax
        from jax.sharding import Mesh, PartitionSpec
        from jax.experimental.shard_map import shard_map
        from concourse import bass2jax

        try:
            cache_dir = os.environ.get(
                "JAX_COMPILATION_CACHE_DIR",
                os.path.join(os.path.expanduser("~"), ".cache", "jax_bass_mha"))
            os.makedirs(cache_dir, exist_ok=True)
            jax.config.update("jax_compilation_cache_dir", cache_dir)
            jax.config.update("jax_persistent_cache_min_compile_time_secs", 10)
        except Exception:
            pass

        self.T = T
        nc = build_nc(T, debug_dump=debug_dump)
        self.nc = nc
        bass2jax.install_neuronx_cc_hook()

        partition_name = (nc.partition_id_tensor.name
                          if nc.partition_id_tensor else None)
        in_names, out_names, out_avals, zero_outs = [], [], [], []
        for alloc in nc.m.functions[0].allocations:
            if not isinstance(alloc, mybir.MemoryLocationSet):
                continue
            name = alloc.memorylocations[0].name
            if alloc.kind == "ExternalInput":
                if name != partition_name:
                    in_names.append(name)
            elif alloc.kind == "ExternalOutput":
                shape = tuple(alloc.tensor_shape)
                dtype = mybir.dt.np(alloc.dtype)
                out_names.append(name)
                out_avals.append(jax.core.ShapedArray(shape, dtype))
                zero_outs.append(np.zeros(shape, dtype))
        self.in_names = list(in_names)
        self.out_names = out_names
        self.out_avals = out_avals
        self.zero_outs = zero_outs
        n_params = len(in_names)
        n_outs = len(out_avals)
        all_in = in_names + out_names
        if partition_name is not None:
            all_in.append(partition_name)

        def _body(*args):
            operands = list(args)
            if partition_name is not None:
                operands.append(bass2jax.partition_id_tensor())
            outs = bass2jax._bass_exec_p.bind(
                *operands,
                out_avals=tuple(out_avals),
                in_names=tuple(all_in),
                out_names=tuple(out_names),
                lowering_input_output_aliases=(),
                sim_require_finite=True,
                sim_require_nnan=True,
                nc=nc,
            )
            return tuple(outs)

        devices = jax.devices()[:NCORES]
        assert len(devices) == NCORES
        mesh = Mesh(np.asarray(devices), ("core",))
        in_specs = (PartitionSpec("core"),) * (n_params + n_outs)
        out_specs = (PartitionSpec("core"),) * n_outs
        donate = tuple(range(n_params, n_params + n_outs))
        self._jitted = jax.jit(
            shard_map(_body, mesh=mesh, in_specs=in_specs,
                      out_specs=out_specs, check_rep=False),
            donate_argnums=donate, keep_unused=True,
        )

    def run(self, in_maps):
        concat_in = [
            np.concatenate([np.asarray(in_maps[c][name]) for c in range(NCORES)],
                           axis=0)
            for name in self.in_names
        ]
        concat_zeros = [
            np.zeros((NCORES * z.shape[0], *z.shape[1:]), z.dtype)
            for z in self.zero_outs
        ]
        out_arrs = self._jitted(*concat_in, *concat_zeros)
        return [
            {
                name: np.asarray(out_arrs[i]).reshape(
                    NCORES, *self.out_avals[i].shape)[c]
                for i, name in enumerate(self.out_names)
            }
            for c in range(NCORES)
        ]


def get_runner(T=2048, debug_dump=False):
    key = (T, debug_dump)
    if key not in _RUNNERS:
        _RUNNERS[key] = _Runner(T, debug_dump)
    return _RUNNERS[key]


def make_in_maps(x, Wq, Wk, Wv, Wp):
    x = np.asarray(x, np.float32)
    Wq = np.asarray(Wq, np.float32)
    Wk = np.asarray(Wk, np.float32)
    Wv = np.asarray(Wv, np.float32)
    Wp = np.asarray(Wp, np.float32)
    xTs = [np.ascontiguousarray(x[b].T) for b in range(x.shape[0])]
    in_maps = []
    for c in range(NCORES):
        b, hg = divmod(c, 4)
        hs = slice(HS * hg, HS * hg + HS)
        in_maps.append({
            "xT": xTs[b],
            "wqT": np.ascontiguousarray(Wq[hs, :].T),
            "wkT": np.ascontiguousarray(Wk[hs, :].T),
            "wvT": np.ascontiguousarray(Wv[hs, :].T),
            "wpT": np.ascontiguousarray(Wp[:, hs].T),
        })
    return in_maps


def kernel(x, Wq, Wk, Wv, Wp, bp):
    x = np.asarray(x, np.float32)
    bp = np.asarray(bp, np.float32)
    Bn, T, Cn = x.shape
    runner = get_runner(T)
    in_maps = make_in_maps(x, Wq, Wk, Wv, Wp)
    results = runner.run(in_maps)
    out = np.empty((Bn, T, Cn), np.float32)
    for b in range(Bn):
        acc = results[4 * b]["out"].astype(np.float32).copy()
        for g in range(1, 4):
            acc += results[4 * b + g]["out"]
        out[b] = acc + bp[None, :]
    return out

